# revision 25
# baseline (speedup 1.0000x reference)
"""Butterfly multiply (n=4096, 12 stages, increasing stride) on 8 Trainium2
NeuronCores.

Math: the 12 butterfly stages factor into
  out = C-blockdiag( permute( A-blockdiag( x^T ) ) )
where stages 0..6 (strides 1..64) compose into 32 dense 128x128 matrices A_o
acting within 128-aligned blocks, and stages 7..11 (strides 128..2048) compose
into 128 dense 32x32 matrices C_i acting across blocks at fixed within-block
index.  Both are composed on the host from the (tiny) twiddle input; the heavy
data (x) runs through two TensorEngine matmul passes per core in fp16.

Device layout (per core, batch shard b=1024):
  pass A:  y1[i', (o,b)] = sum_i A_o[i',i] xT[(o,i), b]     (A stationary)
  permute: z_tau[(a,o), b] = y1[4*tau+a, (o,b)]             (SBUF->SBUF DMA,
           2 KiB contiguous runs -- the whole core batch in one chunk)
  pass B:  psB[(a,o'), b] = sum_(a,o) R'[(a,o),(a,o')] z_tau[(a,o), b]
           with R'[tau] block-diagonal holding C_{4tau+a}^T   (R' stationary)
  store:   n-major (transposed) output; the host untransposes for free.

Host pre/post (not on the device critical path): x is shipped pre-transposed
with 4 o-blocks interleaved per SBUF row (8 KiB DMA rows), output comes back
n-major in tau-groups of 4 (8 KiB DMA rows) and is unscrambled in numpy.

Sharding: batch 8192 split across 8 cores (data parallel), twiddle-derived
matrices replicated.
"""

import numpy as np

LOG_N = 12
N = 4096
BATCH = 8192
N_CORES = 8
B_CORE = BATCH // N_CORES  # 1024 rows per core


def _compose_matrices(twiddle):
    """Compose stages 0..6 -> A (32,128,128) and stages 7..11 -> C (128,32,32),
    in float64."""
    tw = np.asarray(twiddle)[0, 0].astype(np.float64)  # (12, 2048, 2, 2)

    A = np.zeros((32, 128, 128))
    A[:, np.arange(128), np.arange(128)] = 1.0
    for idx in range(7):
        s = 1 << idx
        Ar = A.reshape(32, 128 // (2 * s), 2, s, 128)  # (o, dl, k, j, i_in)
        o = np.arange(32)[:, None, None]
        dl = np.arange(128 // (2 * s))[None, :, None]
        j = np.arange(s)[None, None, :]
        m = (o * (64 // s) + dl) * s + j
        t = tw[idx, m]  # (32, dl, j, 2, 2)
        x0, x1 = Ar[:, :, 0], Ar[:, :, 1]
        new0 = t[..., 0, 0:1] * x0 + t[..., 0, 1:2] * x1
        new1 = t[..., 1, 0:1] * x0 + t[..., 1, 1:2] * x1
        A = np.stack([new0, new1], axis=2).reshape(32, 128, 128)

    C = np.zeros((128, 32, 32))
    C[:, np.arange(32), np.arange(32)] = 1.0
    for idx in range(7, 12):
        s = 1 << idx
        sp = s // 128
        Cr = C.reshape(128, 32 // (2 * sp), 2, sp, 32)  # (i, dl, k, ol, o_in)
        i = np.arange(128)[None, None, :]
        dl = np.arange(32 // (2 * sp))[:, None, None]
        ol = np.arange(sp)[None, :, None]
        m = dl * (128 * sp) + 128 * ol + i  # (dl, ol, i)
        t = np.moveaxis(tw[idx, m], 2, 0)  # (i, dl, ol, 2, 2)
        x0, x1 = Cr[:, :, 0], Cr[:, :, 1]
        new0 = t[..., 0, 0:1] * x0 + t[..., 0, 1:2] * x1
        new1 = t[..., 1, 0:1] * x0 + t[..., 1, 1:2] * x1
        C = np.stack([new0, new1], axis=2).reshape(128, 32, 32)

    # Partition scatter pi: SBUF partition p holds pass-A output row i'(p).
    # i' = 4*tau + a; p = 32*(tau//8) + 8*a + (tau%8).  Each tau's quad is a
    # uniform stride-8 partition set (legal single AP dim) spanning 4 SDMA
    # engine classes (e = 2*((p%32)//4) + p//64) instead of v2's single class.
    i_of_p = np.zeros(128, dtype=np.int64)
    for p in range(128):
        W, a, r = p >> 5, (p >> 3) & 3, p & 7
        i_of_p[p] = 4 * (8 * W + r) + a

    # ATd[k, o*128+p] = A_o[i_of_p[p], k]  (lhsT layout for pass A)
    Aperm = A[:, i_of_p, :]  # rows reordered so matmul output partition p = pi(i')
    ATd = np.ascontiguousarray(np.transpose(Aperm, (0, 2, 1)).transpose(1, 0, 2).reshape(128, 32 * 128))
    # z partition u = a*32+o; R'[u, tau*128 + v=(a*32+o')] = C[4*tau+a][o', o]
    Rd = np.zeros((128, 32, 128))
    for tau in range(32):
        for a in range(4):
            Rd[a * 32:(a + 1) * 32, tau, a * 32:(a + 1) * 32] = C[4 * tau + a].T
    Rd = np.ascontiguousarray(Rd.reshape(128, 32 * 128))
    return ATd, Rd


def _build_program(b_core=B_CORE):
    """Trace + compile the per-core Bass program. Returns nc."""
    import concourse.bacc as bacc
    import concourse.tile as tile
    import concourse.mybir as mybir
    from contextlib import ExitStack

    f32 = mybir.dt.float32
    dt = mybir.dt.float16
    B = b_core  # 1024

    nc = bacc.Bacc(
        "TRN2",
        target_bir_lowering=False,
        debug=False,
        enable_asserts=False,
        num_devices=1,
    )
    # x shipped pre-transposed, 4 o-blocks per row: [og, i, (o4, b)]
    x_ap = nc.dram_tensor("xt", (8, 128, 4 * B), dt, kind="ExternalInput").ap()
    at_ap = nc.dram_tensor("AT", (128, 32 * 128), dt, kind="ExternalInput").ap()
    r_ap = nc.dram_tensor("R", (128, 32 * 128), dt, kind="ExternalInput").ap()
    # output n-major in tau-groups: [tg, v, (t4, b)]
    y_ap = nc.dram_tensor("y", (8, 128, 4 * B), dt, kind="ExternalOutput").ap()

    with tile.TileContext(nc) as tc, ExitStack() as ctx:
        wpool = ctx.enter_context(tc.tile_pool(name="weights", bufs=1))
        xpool = ctx.enter_context(tc.tile_pool(name="xg", bufs=3))
        y1pool = ctx.enter_context(tc.tile_pool(name="y1", bufs=1))
        zpool = ctx.enter_context(tc.tile_pool(name="z", bufs=10))
        opool = ctx.enter_context(tc.tile_pool(name="outT", bufs=3))
        psA_pool = ctx.enter_context(tc.tile_pool(name="psA", bufs=4, space="PSUM"))
        psB_pool = ctx.enter_context(tc.tile_pool(name="psB", bufs=4, space="PSUM"))

        ATw = wpool.tile([128, 32 * 128], dt, tag="ATw")
        Rw = wpool.tile([128, 32 * 128], dt, tag="Rw")
        nc.sync.dma_start(ATw[:], at_ap)
        nc.scalar.dma_start(Rw[:], r_ap)

        # ---- phase A: per o-group load (8 KiB rows), per-o matmul + cast
        y1 = y1pool.tile([128, 32 * B], dt, tag="y1")  # [p=pi(i'), (o, b)]
        for og in range(8):
            xt = xpool.tile([128, 4 * B], dt, tag="xg")
            leng = nc.sync if (og % 2 == 0) else nc.scalar
            leng.dma_start(xt[:], x_ap[og])
            for o4 in range(4):
                o = og * 4 + o4
                for h in range(2):
                    psA = psA_pool.tile([128, 512], f32, tag="psA")
                    nc.tensor.matmul(
                        psA[:],
                        ATw[:, o * 128:(o + 1) * 128],
                        xt[:, o4 * B + h * 512:o4 * B + (h + 1) * 512],
                        start=True,
                        stop=True,
                    )
                    dst = y1[:, o * B + h * 512:o * B + (h + 1) * 512]
                    if o % 2 == 0:
                        nc.vector.tensor_copy(dst, psA[:])
                    else:
                        nc.scalar.copy(dst, psA[:])

        # ---- phase B: per-tau permute DMA (src = stride-8 partition quad,
        #      4 engine classes) -> matmul (R' stationary) -> cast -> store
        #      every 4 processed taus.  Process order rotates the 4 disjoint
        #      engine-class sets; perms round-robin over 3 DMA rings.
        # tau = 8*W + r lives on partitions {32*W + 8*a + r}.
        order = []
        sets = [[8 * W + r for W in range(4) for r in range(8)
                 if (W // 2) * 2 + (r // 4) == s] for s in range(4)]
        for j in range(8):
            for s in range(4):
                order.append(sets[s][j])
        y1v = y1[:].rearrange("(w a r) (o b) -> w r a o b", w=4, a=4, r=8, b=B)
        for step, tau in enumerate(order):
            W, r = tau // 8, tau % 8
            z = zpool.tile([128, B], dt, tag="z")
            peng = nc.gpsimd if (step % 8 in (2, 5, 7)) else nc.sync
            peng.dma_start(z[:], y1v[W, r])
            if step % 4 == 0:
                outT = opool.tile([128, 4 * B], dt, tag="outT")
            for h in range(2):
                psB = psB_pool.tile([128, 512], f32, tag="psB")
                nc.tensor.matmul(
                    psB[:],
                    Rw[:, tau * 128:(tau + 1) * 128],
                    z[:, h * 512:(h + 1) * 512],
                    start=True,
                    stop=True,
                )
                dst = outT[:, (step % 4) * B + h * 512:(step % 4) * B + (h + 1) * 512]
                if h == 0:
                    nc.vector.tensor_copy(dst, psB[:])
                else:
                    nc.scalar.copy(dst, psB[:])
            if step % 4 == 3:
                nc.scalar.dma_start(y_ap[step // 4], outT[:])

    nc.compile()
    return nc


_CACHE = {}


def _get_program():
    if "nc" not in _CACHE:
        _CACHE["nc"] = _build_program()
    return _CACHE["nc"]


def run(x, twiddle, trace=False, trace_kwargs=None):
    """Run the butterfly kernel on 8 cores. Returns (out, BassKernelResults)."""
    from concourse.bass_utils import run_bass_kernel_spmd

    nc = _get_program()
    np_dt = np.float16

    ATd, Rd = _compose_matrices(twiddle)
    ATd = ATd.astype(np_dt)
    Rd = Rd.astype(np_dt)

    x = np.asarray(x)
    in_dtype = x.dtype
    xd = x.astype(np_dt)

    in_maps = []
    for c in range(N_CORES):
        shard = xd[c * B_CORE:(c + 1) * B_CORE]  # [b, n]
        # [og, i, o4, b]: n = og*512 + o4*128 + i
        xtc = np.ascontiguousarray(
            shard.reshape(B_CORE, 8, 4, 128).transpose(1, 3, 2, 0)
        ).reshape(8, 128, 4 * B_CORE)
        in_maps.append({"xt": xtc, "AT": ATd, "R": Rd})

    res = run_bass_kernel_spmd(
        nc,
        in_maps,
        core_ids=list(range(N_CORES)),
        trace=trace,
        **(trace_kwargs or {}),
    )
    # store-group g holds processed taus order[4g..4g+4)
    order = []
    sets = [[8 * W + r for W in range(4) for r in range(8)
             if (W // 2) * 2 + (r // 4) == s] for s in range(4)]
    for j in range(8):
        for s in range(4):
            order.append(sets[s][j])
    outs = []
    for r in res.results:
        yd = r["y"].reshape(8, 4, 32, 4, B_CORE)  # [g, a, o', idx, b]
        oc = np.empty((B_CORE, 32, 32, 4), dtype=yd.dtype)  # [b, o', tau, a]
        for g in range(8):
            for idx in range(4):
                tau = order[4 * g + idx]
                oc[:, :, tau, :] = yd[g, :, :, idx, :].transpose(2, 1, 0)
        outs.append(oc.reshape(B_CORE, N))
    out = np.concatenate(outs, axis=0)
    return out.astype(in_dtype), res


def kernel(x, twiddle):
    out, _ = run(x, twiddle)
    return out
